# revision 23
# baseline (speedup 1.0000x reference)
"""Blended-MoE 3-layer MLP (moe_routing) on 8 trn2 NeuronCores.

Math: per layer  z[b,o] = sum_e blend[e,b] * (w[e] @ h[b] + bias[e])[o],
ELU between layers.  Single contraction per layer:

    z[b,o] = sum_{(e,i)} (blend[e,b] * hT[i,b]) * wT[(e,i), o]
           + sum_e blend[e,b] * bias[e,o]          (bias via one K=8 matmul)

Data-parallel across 8 cores (128 batch rows each); expert weights are
replicated, host-side pre-transposed into SBUF-image layout.

The kernel is HBM-bound: ~10.8 MB of fp16 weights per core at ~390-450
GB/s on the sync HWDGE queue.  Schedule notes (hard-won on HW traces):
  - ONE merged head transfer (xt image + identity + blend broadcast)
    leads the stream, then weights in strict consumption order; every
    transfer spans all 128 partitions (a 96-partition transfer was seen
    to permanently imbalance the per-engine DMA FIFOs, and the resulting
    straggle on the LAST transfer gated all of layer 2)
  - the blend broadcast is only [128, E*128]; the he expansion is a
    single wide DVE op per layer using 0-stride broadcast APs, into an
    (i-tile, e, b) column layout so boundary expansions write contiguous
    1024-col slices
  - ELU uses exp(min(z,0)) = min(exp(z),1): Exp and Relu on ACT straight
    from PSUM, two full-width DVE ops, then per-128-col transpose whose
    result is read directly from PSUM by the expansion
  - layers 1/2 consume k-tiles it-major so each transposed quarter
    unlocks 8 matmuls as soon as its expansion lands
  - the PE frequency ramps (~50%% -> 100%% over ~4-6 us of continuous
    work) and resets on ANY idle: dummy "warm" matmuls on resident data
    bridge the pre-layer-0 window, and fp32 warms that READ the ELU
    intermediates (so the list scheduler cannot hoist them earlier)
    keep the ramp alive through each boundary chain
  - weight-group granularity is the matmul batching unit (per-group DMA
    sems batch the PE into long bursts); the final w2 groups are small
    so little compute trails the last weight byte
"""

import numpy as np

import concourse.bass as bass
import concourse.mybir as mybir
import concourse.tile as tile
from concourse.bass_utils import run_bass_kernel_spmd

import bass_rust

# ---- config ----------------------------------------------------------------
N_CORES = 8
B, E = 1024, 8
DIN, D1, D2, D3 = 480, 512, 512, 311
O2A = 160                       # layer-2 output split: [0:160), [160:311)
O2B = D3 - O2A

PROFILE = {"trace": False, "tmpdir": None}
LAST_RESULT = [None]

_NC_CACHE = {}
_SPLIT_N = [0]

# weight-group tiling (number of k-tiles per DMA transfer)
W0_GROUPS = [4, 8, 10, 6, 2, 2]  # it-major (it, e), 32 k-tiles (it=3 zero-padded)
W1_GROUPS = [8, 8, 8, 4, 2, 2]  # it-major, small head + tapered tail
W2_GROUPS = [16, 12, 2, 2]      # last groups kept small for a short tail


def _locate(groups, j):
    """k-tile j -> (group index, offset within group)."""
    g = 0
    while j >= groups[g]:
        j -= groups[g]
        g += 1
    return g, j


def _order1():
    """(e, it) consumption order for layers 1/2: it-major, so each
    boundary quarter (one transposed 128-col block of h) unlocks 8
    matmuls as soon as its expansion lands."""
    return [(e, it) for it in range(4) for e in range(E)]


def _split_multi_waits(nc, max_waits=1):
    """This container's walrus only supports one sync-wait command per
    instruction; spill extras onto same-engine NOPs inserted just before."""
    for f in nc.m.functions:
        for bb in f.blocks:
            insts = bb.instructions
            i = 0
            while i < len(insts):
                inst = insts[i]
                si = inst.sync_info
                if si is not None and len(si.on_wait) > max_waits:
                    waits = list(si.on_wait)
                    extra, keep = waits[:-max_waits], waits[-max_waits:]
                    for w in extra:
                        _SPLIT_N[0] += 1
                        nop = mybir.InstNoOp(
                            name=f"wsplit-{_SPLIT_N[0]}", ins=[], outs=[]
                        )
                        nop.engine = inst.engine
                        nop.sync_info = bass_rust.SyncInfo(
                            on_wait=[w], on_update=[]
                        )
                        insts.insert(i, nop)
                        i += 1
                    inst.sync_info = bass_rust.SyncInfo(
                        on_wait=keep, on_update=list(si.on_update)
                    )
                i += 1


class _FastTailTC(tile.TileContext):
    """Tile's kernel tail is drain-with-per-sem-waits + 2 all-engine
    barriers + per-sem clears; the per-sem waits explode into ~70 NOPs per
    engine under the single-wait walrus (~8 us).  All DMAs except the
    output writeback have already been observed by their consumers, so a
    barrier (engines idle, all triggers issued) followed by the range-based
    DMA drain + semaphore clear inside clear_and_free_semaphores is enough."""

    def _drain_and_barrier(self, tick_clock, wait_clock):
        nc = self.nc
        nc.all_engine_barrier()
        popped = nc._tile_sem_poison_stack.pop()
        assert popped is self._sem_poison
        assert self.sems is not None
        nc.clear_and_free_semaphores(list(self.sems.allocated().values()))


def _build_nc():
    f32 = mybir.dt.float32
    dt = mybir.dt.float16
    nc = bass.Bass()

    # ---- DRAM tensors ----
    # head: [xt image (it,b) | 128x128 identity | bbsm (e,b) broadcast]
    head_d = nc.dram_tensor(
        "head", [128, 512 + 128 + E * 128], dt, kind="ExternalInput"
    )
    # small: [ blT (8x128) | bias0 (8x512) | bias1 (8x512) | bias2 (8x311) ]
    SMALL_COLS = 128 + D1 + D2 + D3
    small_d = nc.dram_tensor("small", [8, SMALL_COLS], dt, kind="ExternalInput")

    w0_d = nc.dram_tensor("w0", [128, 32 * D1], dt, kind="ExternalInput")
    w1_d = nc.dram_tensor("w1", [128, 32 * D2], dt, kind="ExternalInput")
    w2_d = nc.dram_tensor("w2", [128, 32 * D3], dt, kind="ExternalInput")
    out_d = nc.dram_tensor("out", [128, D3], f32, kind="ExternalOutput")

    with _FastTailTC(nc) as tc:
        with (
            tc.tile_pool(name="const", bufs=1) as const,
            tc.tile_pool(name="w", bufs=16) as wpool,
            tc.tile_pool(name="acts", bufs=2) as acts,
            tc.tile_pool(name="tmp", bufs=2) as tmp,
            tc.tile_pool(name="zp", bufs=2, space="PSUM") as zp,
            tc.tile_pool(name="zo", bufs=1, space="PSUM") as zo,
            tc.tile_pool(name="tp", bufs=2, space="PSUM") as tp,
        ):
            # ---- inputs lead the sync HWDGE stream (SWDGE starts too late
            # and its Q7 launches contend with the SDMA engines) ----
            head_sb = const.tile([128, 512 + 128 + E * 128], dt)
            nc.sync.dma_start(head_sb[:], head_d[:])
            small_sb = const.tile([8, SMALL_COLS], dt)
            nc.sync.dma_start(small_sb[:], small_d[:])
            xti_sb = head_sb[:, 0:640]
            bbsm = head_sb[:, 640 : 640 + E * 128]

            ident = xti_sb[:, 512:640]
            bl8 = small_sb[:, 0:128]
            waug = []
            off = 128
            for n in (D1, D2, D3):
                waug.append(small_sb[:, off : off + n])
                off += n

            # ---- weight stream: strict consumption order on sync HWDGE ----
            w0 = []
            c = 0
            for g, nt in enumerate(W0_GROUPS):
                t = wpool.tile([128, nt * D1], dt, tag=f"w0{g}", bufs=1)
                nc.sync.dma_start(t[:], w0_d[:, c * D1 : (c + nt) * D1])
                w0.append(t)
                c += nt
            w1 = []
            c = 0
            for g, nt in enumerate(W1_GROUPS):
                t = wpool.tile([128, nt * D2], dt, tag=f"w1{g}", bufs=1)
                nc.sync.dma_start(t[:], w1_d[:, c * D2 : (c + nt) * D2])
                w1.append(t)
                c += nt
            w2 = []
            c = 0
            for g, nt in enumerate(W2_GROUPS):
                t = wpool.tile([128, nt * D3], dt, tag=f"w2{g}", bufs=1)
                nc.sync.dma_start(t[:], w2_d[:, c * D3 : (c + nt) * D3])
                w2.append(t)
                c += nt

            bb_e_b = bbsm.rearrange("p (e b) -> p e b", e=E)

            def expand(he_dst, src, n_it, it0=0):
                # he[:, e*512 + (it0+it)*128 + b] = src[p, (it, b)] * blend[e, b]
                # single wide DVE op: both operands broadcast (e / it axes)
                out = he_dst[:].rearrange("p (e it b) -> p e it b", e=E, it=4)[
                    :, :, it0 : it0 + n_it, :
                ]
                nc.vector.tensor_tensor(
                    out,
                    src.rearrange("p (it b) -> p it b", it=n_it)[
                        :, None, :, :
                    ].broadcast_to([128, E, n_it, 128]),
                    bb_e_b[:, :, None, :].broadcast_to([128, E, n_it, 128]),
                    mybir.AluOpType.mult,
                )

            # ---- he for layer 0; layout he[p, (it, e, b)]: one expansion
            # call per i-tile so the first matmuls unlock ~2us earlier ----
            he = acts.tile([128, E * 512], dt, tag="he")
            for it in range(4):
                nc.vector.tensor_tensor(
                    he[:, it * 1024 : (it + 1) * 1024].rearrange(
                        "p (e b) -> p e b", e=E
                    ),
                    xti_sb[:, it * 128 : (it + 1) * 128][
                        :, None, :
                    ].broadcast_to([128, E, 128]),
                    bb_e_b,
                    mybir.AluOpType.mult,
                )

            scr = tp.tile([128, 512], f32, tag="scr", bufs=1)

            def warm(n):
                # Dummy matmuls on resident data: keep the PE busy (and its
                # frequency ramp alive) while a boundary chain runs.
                for _ in range(n):
                    nc.tensor.matmul(
                        scr[:], ident, xti_sb[:, 0:512], start=True, stop=True
                    )

            # ---- layer 0 ----
            z0 = zp.tile([128, D1], f32, tag="z")
            warm(5)
            nc.tensor.matmul(z0[:], bl8, waug[0], start=True, stop=False)
            j = 0
            for g, t in enumerate(w0):
                for loc in range(W0_GROUPS[g]):
                    it, e = divmod(j, E)
                    nc.tensor.matmul(
                        z0[:],
                        he[:, it * 1024 + e * 128 : it * 1024 + (e + 1) * 128],
                        t[:, loc * D1 : (loc + 1) * D1],
                        start=False,
                        stop=(j == 31),
                    )
                    j += 1

            def boundary(z, he_next, tag):
                """ELU via exp(min(z,0)) = min(exp(z),1):
                   ex2 = Exp(z), relu = Relu(z)      (ACT, straight from PSUM)
                   t1 = min(ex2,1) - 1; hh = t1+relu (DVE, full width)
                then transpose + expansion per 128-col quarter (expansion
                reads the transpose result straight from PSUM).  fp32 warm
                matmuls read ex2, so they become ready exactly when the
                boundary starts and keep the PE frequency ramp alive."""
                n = z.shape[1]
                ex2 = tmp.tile([128, n], f32, tag=f"{tag}x")
                relu = tmp.tile([128, n], dt, tag=f"{tag}r")
                t1 = tmp.tile([128, n], dt, tag=f"{tag}t")
                hh = tmp.tile([128, n], dt, tag=f"{tag}h")
                # quarter 0 narrow (shortest path to the first transpose),
                # then the remaining 384 cols in one wide pass
                for lo, hi in ((0, 128), (128, n)):
                    nc.scalar.activation(
                        ex2[:, lo:hi], z[:, lo:hi],
                        mybir.ActivationFunctionType.Exp,
                    )
                    nc.vector.tensor_scalar(
                        t1[:, lo:hi], ex2[:, lo:hi], 1.0, -1.0,
                        mybir.AluOpType.min, mybir.AluOpType.add,
                    )
                    nc.scalar.activation(
                        relu[:, lo:hi], z[:, lo:hi],
                        mybir.ActivationFunctionType.Relu,
                    )
                    nc.vector.tensor_tensor(
                        hh[:, lo:hi], t1[:, lo:hi], relu[:, lo:hi],
                        mybir.AluOpType.add,
                    )
                    if lo == 0:
                        for _ in range(3):
                            nc.tensor.matmul(
                                scr[:, 0:128], ex2[:, 0:128], ex2[:, 0:128],
                                start=True, stop=True,
                            )
                for q in range(4):
                    tps = tp.tile([128, 128], dt, tag=f"t{q % 2}", bufs=1)
                    nc.tensor.transpose(
                        tps[:], hh[:, q * 128 : (q + 1) * 128], ident
                    )
                    nc.vector.tensor_tensor(
                        he_next[:, q * 1024 : (q + 1) * 1024].rearrange(
                            "p (e b) -> p e b", e=E
                        ),
                        tps[:][:, None, :].broadcast_to([128, E, 128]),
                        bb_e_b,
                        mybir.AluOpType.mult,
                    )
                    yield q

            # layer 1
            he1 = acts.tile([128, E * 512], dt, tag="he")
            z1 = zp.tile([128, D2], f32, tag="z")
            nc.tensor.matmul(z1[:], bl8, waug[1], start=True, stop=False)
            order = _order1()
            bgen = boundary(z0, he1, "b0")
            next(bgen)
            next(bgen)
            for j, (e, it) in enumerate(order):
                if j in (8, 16):
                    next(bgen)  # queue quarter q+1 behind this it-block
                g, loc = _locate(W1_GROUPS, j)
                nc.tensor.matmul(
                    z1[:],
                    he1[:, it * 1024 + e * 128 : it * 1024 + (e + 1) * 128],
                    w1[g][:, loc * D2 : (loc + 1) * D2],
                    start=False,
                    stop=(j == 31),
                )

            # layer 2
            he2 = acts.tile([128, E * 512], dt, tag="he")
            z2 = zo.tile([128, D3], f32, tag="z2")
            nc.tensor.matmul(z2[:], bl8, waug[2], start=True, stop=False)
            bgen = boundary(z1, he2, "b1")
            next(bgen)
            next(bgen)
            for j, (e, it) in enumerate(order):
                if j in (8, 16):
                    next(bgen)
                g, loc = _locate(W2_GROUPS, j)
                nc.tensor.matmul(
                    z2[:],
                    he2[:, it * 1024 + e * 128 : it * 1024 + (e + 1) * 128],
                    w2[g][:, loc * D3 : (loc + 1) * D3],
                    start=False,
                    stop=(j == 31),
                )
            out_sb = tmp.tile([128, D3], f32, tag="osb")
            nc.vector.tensor_copy(out_sb[:, 0:156], z2[:, 0:156])
            nc.scalar.copy(out_sb[:, 156:D3], z2[:, 156:D3])
            nc.sync.dma_start(out_d[:], out_sb[:])

    _split_multi_waits(nc)
    return nc


# ---- host-side packing -----------------------------------------------------


def _wimgs(w0, w1, w2, np_dt):
    wt = [np.ascontiguousarray(w.transpose(0, 2, 1)) for w in (w0, w1, w2)]

    # layer 0: it-major over (it, e), it=3 rows 96:128 zero-padded
    w0i = np.zeros((128, 32 * D1), np.float32)
    j = 0
    for it in range(4):
        for e in range(E):
            rows = 128 if it < 3 else 96
            w0i[:rows, j * D1 : (j + 1) * D1] = wt[0][e][
                it * 128 : it * 128 + rows
            ]
            j += 1

    order = _order1()
    w1i = np.empty((128, 32 * D2), np.float32)
    for j, (e, it) in enumerate(order):
        w1i[:, j * D2 : (j + 1) * D2] = wt[1][e][it * 128 : (it + 1) * 128]
    w2i = np.empty((128, 32 * D3), np.float32)
    for j, (e, it) in enumerate(order):
        w2i[:, j * D3 : (j + 1) * D3] = wt[2][e][it * 128 : (it + 1) * 128]
    return {
        "w0": np.ascontiguousarray(w0i).astype(np_dt),
        "w1": np.ascontiguousarray(w1i).astype(np_dt),
        "w2": np.ascontiguousarray(w2i).astype(np_dt),
    }


def kernel(x, weight_blend, w0, b0, w1, b1, w2, b2):
    np_dt = np.float16

    if "nc" not in _NC_CACHE:
        _NC_CACHE["nc"] = _build_nc()
    nc = _NC_CACHE["nc"]

    x = np.asarray(x, np.float32)
    weight_blend = np.asarray(weight_blend, np.float32)
    wimgs = _wimgs(np.asarray(w0), np.asarray(w1), np.asarray(w2), np_dt)
    biases = [np.asarray(b, np.float32) for b in (b0, b1, b2)]
    eye = np.eye(128, dtype=np.float32)

    bc = B // N_CORES
    in_maps = []
    for c in range(N_CORES):
        sl = slice(c * bc, (c + 1) * bc)
        xT = np.zeros((4 * 128, bc), np.float32)
        xT[:DIN] = x[sl].T
        xt_img = xT.reshape(4, 128, bc).transpose(1, 0, 2).reshape(128, 4 * bc)
        bl = weight_blend[:, sl]  # (8, 128)
        small_img = np.concatenate([bl] + biases, axis=1)
        bbsm = np.broadcast_to(bl[None, :, :], (128, E, bc)).reshape(128, E * bc)
        head = np.concatenate([xt_img, eye, bbsm], axis=1)
        in_maps.append(
            {
                **wimgs,
                "head": np.ascontiguousarray(head).astype(np_dt),
                "small": np.ascontiguousarray(small_img).astype(np_dt),
            }
        )

    res = run_bass_kernel_spmd(
        nc,
        in_maps,
        core_ids=list(range(N_CORES)),
        trace=PROFILE["trace"],
        tmpdir=PROFILE["tmpdir"],
    )
    LAST_RESULT[0] = res
    return np.concatenate(
        [res.results[c]["out"] for c in range(N_CORES)], axis=0
    )


# revision 24
# speedup vs baseline: 1.0026x; 1.0026x over previous
"""Blended-MoE 3-layer MLP (moe_routing) on 8 trn2 NeuronCores.

Math: per layer  z[b,o] = sum_e blend[e,b] * (w[e] @ h[b] + bias[e])[o],
ELU between layers.  Single contraction per layer:

    z[b,o] = sum_{(e,i)} (blend[e,b] * hT[i,b]) * wT[(e,i), o]
           + sum_e blend[e,b] * bias[e,o]          (bias via one K=8 matmul)

Data-parallel across 8 cores (128 batch rows each); expert weights are
replicated, host-side pre-transposed into SBUF-image layout.

The kernel is HBM-bound: ~10.8 MB of fp16 weights per core at ~390-450
GB/s on the sync HWDGE queue.  Schedule notes (hard-won on HW traces):
  - ONE merged head transfer (xt image + identity + blend broadcast)
    leads the stream, then weights in strict consumption order; every
    transfer spans all 128 partitions (a 96-partition transfer was seen
    to permanently imbalance the per-engine DMA FIFOs, and the resulting
    straggle on the LAST transfer gated all of layer 2)
  - the blend broadcast is only [128, E*128]; the he expansion is a
    single wide DVE op per layer using 0-stride broadcast APs, into an
    (i-tile, e, b) column layout so boundary expansions write contiguous
    1024-col slices
  - ELU uses exp(min(z,0)) = min(exp(z),1): Exp and Relu on ACT straight
    from PSUM, two full-width DVE ops, then per-128-col transpose whose
    result is read directly from PSUM by the expansion
  - layers 1/2 consume k-tiles it-major so each transposed quarter
    unlocks 8 matmuls as soon as its expansion lands
  - the PE frequency ramps (~50%% -> 100%% over ~4-6 us of continuous
    work) and resets on ANY idle: dummy "warm" matmuls on resident data
    bridge the pre-layer-0 window, and fp32 warms that READ the ELU
    intermediates (so the list scheduler cannot hoist them earlier)
    keep the ramp alive through each boundary chain
  - weight-group granularity is the matmul batching unit (per-group DMA
    sems batch the PE into long bursts); the final w2 groups are small
    so little compute trails the last weight byte
"""

import numpy as np

import concourse.bass as bass
import concourse.mybir as mybir
import concourse.tile as tile
from concourse.bass_utils import run_bass_kernel_spmd

import bass_rust

# ---- config ----------------------------------------------------------------
N_CORES = 8
B, E = 1024, 8
DIN, D1, D2, D3 = 480, 512, 512, 311
O2A = 160                       # layer-2 output split: [0:160), [160:311)
O2B = D3 - O2A

PROFILE = {"trace": False, "tmpdir": None}
LAST_RESULT = [None]

_NC_CACHE = {}
_SPLIT_N = [0]

# weight-group tiling (number of k-tiles per DMA transfer)
W0_GROUPS = [4, 8, 10, 6, 4]    # it-major (it, e), 32 k-tiles (it=3 zero-padded)
W1_GROUPS = [8, 8, 8, 8]        # it-major, small head group starts L1 early
W2_GROUPS = [16, 12, 2, 2]      # last groups kept small for a short tail


def _locate(groups, j):
    """k-tile j -> (group index, offset within group)."""
    g = 0
    while j >= groups[g]:
        j -= groups[g]
        g += 1
    return g, j


def _order1():
    """(e, it) consumption order for layers 1/2: it-major, so each
    boundary quarter (one transposed 128-col block of h) unlocks 8
    matmuls as soon as its expansion lands."""
    return [(e, it) for it in range(4) for e in range(E)]


def _split_multi_waits(nc, max_waits=1):
    """This container's walrus only supports one sync-wait command per
    instruction; spill extras onto same-engine NOPs inserted just before."""
    for f in nc.m.functions:
        for bb in f.blocks:
            insts = bb.instructions
            i = 0
            while i < len(insts):
                inst = insts[i]
                si = inst.sync_info
                if si is not None and len(si.on_wait) > max_waits:
                    waits = list(si.on_wait)
                    extra, keep = waits[:-max_waits], waits[-max_waits:]
                    for w in extra:
                        _SPLIT_N[0] += 1
                        nop = mybir.InstNoOp(
                            name=f"wsplit-{_SPLIT_N[0]}", ins=[], outs=[]
                        )
                        nop.engine = inst.engine
                        nop.sync_info = bass_rust.SyncInfo(
                            on_wait=[w], on_update=[]
                        )
                        insts.insert(i, nop)
                        i += 1
                    inst.sync_info = bass_rust.SyncInfo(
                        on_wait=keep, on_update=list(si.on_update)
                    )
                i += 1


class _FastTailTC(tile.TileContext):
    """Tile's kernel tail is drain-with-per-sem-waits + 2 all-engine
    barriers + per-sem clears; the per-sem waits explode into ~70 NOPs per
    engine under the single-wait walrus (~8 us).  All DMAs except the
    output writeback have already been observed by their consumers, so a
    barrier (engines idle, all triggers issued) followed by the range-based
    DMA drain + semaphore clear inside clear_and_free_semaphores is enough."""

    def _drain_and_barrier(self, tick_clock, wait_clock):
        nc = self.nc
        nc.all_engine_barrier()
        popped = nc._tile_sem_poison_stack.pop()
        assert popped is self._sem_poison
        assert self.sems is not None
        nc.clear_and_free_semaphores(list(self.sems.allocated().values()))


def _build_nc():
    f32 = mybir.dt.float32
    dt = mybir.dt.float16
    nc = bass.Bass()

    # ---- DRAM tensors ----
    # head: [xt image (it,b) | 128x128 identity | bbsm (e,b) broadcast]
    head_d = nc.dram_tensor(
        "head", [128, 512 + 128 + E * 128], dt, kind="ExternalInput"
    )
    # small: [ blT (8x128) | bias0 (8x512) | bias1 (8x512) | bias2 (8x311) ]
    SMALL_COLS = 128 + D1 + D2 + D3
    small_d = nc.dram_tensor("small", [8, SMALL_COLS], dt, kind="ExternalInput")

    w0_d = nc.dram_tensor("w0", [128, 32 * D1], dt, kind="ExternalInput")
    w1_d = nc.dram_tensor("w1", [128, 32 * D2], dt, kind="ExternalInput")
    w2_d = nc.dram_tensor("w2", [128, 32 * D3], dt, kind="ExternalInput")
    out_d = nc.dram_tensor("out", [128, D3], f32, kind="ExternalOutput")

    with _FastTailTC(nc) as tc:
        with (
            tc.tile_pool(name="const", bufs=1) as const,
            tc.tile_pool(name="w", bufs=16) as wpool,
            tc.tile_pool(name="acts", bufs=2) as acts,
            tc.tile_pool(name="tmp", bufs=2) as tmp,
            tc.tile_pool(name="zp", bufs=2, space="PSUM") as zp,
            tc.tile_pool(name="zo", bufs=1, space="PSUM") as zo,
            tc.tile_pool(name="tp", bufs=2, space="PSUM") as tp,
        ):
            # ---- inputs lead the sync HWDGE stream (SWDGE starts too late
            # and its Q7 launches contend with the SDMA engines) ----
            head_sb = const.tile([128, 512 + 128 + E * 128], dt)
            nc.sync.dma_start(head_sb[:], head_d[:])
            small_sb = const.tile([8, SMALL_COLS], dt)
            nc.sync.dma_start(small_sb[:], small_d[:])
            xti_sb = head_sb[:, 0:640]
            bbsm = head_sb[:, 640 : 640 + E * 128]

            ident = xti_sb[:, 512:640]
            bl8 = small_sb[:, 0:128]
            waug = []
            off = 128
            for n in (D1, D2, D3):
                waug.append(small_sb[:, off : off + n])
                off += n

            # ---- weight stream: strict consumption order on sync HWDGE ----
            w0 = []
            c = 0
            for g, nt in enumerate(W0_GROUPS):
                t = wpool.tile([128, nt * D1], dt, tag=f"w0{g}", bufs=1)
                nc.sync.dma_start(t[:], w0_d[:, c * D1 : (c + nt) * D1])
                w0.append(t)
                c += nt
            w1 = []
            c = 0
            for g, nt in enumerate(W1_GROUPS):
                t = wpool.tile([128, nt * D2], dt, tag=f"w1{g}", bufs=1)
                nc.sync.dma_start(t[:], w1_d[:, c * D2 : (c + nt) * D2])
                w1.append(t)
                c += nt
            w2 = []
            c = 0
            for g, nt in enumerate(W2_GROUPS):
                t = wpool.tile([128, nt * D3], dt, tag=f"w2{g}", bufs=1)
                nc.sync.dma_start(t[:], w2_d[:, c * D3 : (c + nt) * D3])
                w2.append(t)
                c += nt

            bb_e_b = bbsm.rearrange("p (e b) -> p e b", e=E)

            def expand(he_dst, src, n_it, it0=0):
                # he[:, e*512 + (it0+it)*128 + b] = src[p, (it, b)] * blend[e, b]
                # single wide DVE op: both operands broadcast (e / it axes)
                out = he_dst[:].rearrange("p (e it b) -> p e it b", e=E, it=4)[
                    :, :, it0 : it0 + n_it, :
                ]
                nc.vector.tensor_tensor(
                    out,
                    src.rearrange("p (it b) -> p it b", it=n_it)[
                        :, None, :, :
                    ].broadcast_to([128, E, n_it, 128]),
                    bb_e_b[:, :, None, :].broadcast_to([128, E, n_it, 128]),
                    mybir.AluOpType.mult,
                )

            # ---- he for layer 0; layout he[p, (it, e, b)]: one expansion
            # call per i-tile so the first matmuls unlock ~2us earlier ----
            he = acts.tile([128, E * 512], dt, tag="he")
            for it in range(4):
                nc.vector.tensor_tensor(
                    he[:, it * 1024 : (it + 1) * 1024].rearrange(
                        "p (e b) -> p e b", e=E
                    ),
                    xti_sb[:, it * 128 : (it + 1) * 128][
                        :, None, :
                    ].broadcast_to([128, E, 128]),
                    bb_e_b,
                    mybir.AluOpType.mult,
                )

            scr = tp.tile([128, 512], f32, tag="scr", bufs=1)

            def warm(n):
                # Dummy matmuls on resident data: keep the PE busy (and its
                # frequency ramp alive) while a boundary chain runs.
                for _ in range(n):
                    nc.tensor.matmul(
                        scr[:], ident, xti_sb[:, 0:512], start=True, stop=True
                    )

            # ---- layer 0 ----
            z0 = zp.tile([128, D1], f32, tag="z")
            warm(5)
            nc.tensor.matmul(z0[:], bl8, waug[0], start=True, stop=False)
            j = 0
            for g, t in enumerate(w0):
                for loc in range(W0_GROUPS[g]):
                    it, e = divmod(j, E)
                    nc.tensor.matmul(
                        z0[:],
                        he[:, it * 1024 + e * 128 : it * 1024 + (e + 1) * 128],
                        t[:, loc * D1 : (loc + 1) * D1],
                        start=False,
                        stop=(j == 31),
                    )
                    j += 1

            def boundary(z, he_next, tag):
                """ELU via exp(min(z,0)) = min(exp(z),1):
                   ex2 = Exp(z), relu = Relu(z)      (ACT, straight from PSUM)
                   t1 = min(ex2,1) - 1; hh = t1+relu (DVE, full width)
                then transpose + expansion per 128-col quarter (expansion
                reads the transpose result straight from PSUM).  fp32 warm
                matmuls read ex2, so they become ready exactly when the
                boundary starts and keep the PE frequency ramp alive."""
                n = z.shape[1]
                ex2 = tmp.tile([128, n], f32, tag=f"{tag}x")
                relu = tmp.tile([128, n], dt, tag=f"{tag}r")
                t1 = tmp.tile([128, n], dt, tag=f"{tag}t")
                hh = tmp.tile([128, n], dt, tag=f"{tag}h")
                # quarter 0 narrow (shortest path to the first transpose),
                # then the remaining 384 cols in one wide pass
                for lo, hi in ((0, 128), (128, n)):
                    nc.scalar.activation(
                        ex2[:, lo:hi], z[:, lo:hi],
                        mybir.ActivationFunctionType.Exp,
                    )
                    nc.vector.tensor_scalar(
                        t1[:, lo:hi], ex2[:, lo:hi], 1.0, -1.0,
                        mybir.AluOpType.min, mybir.AluOpType.add,
                    )
                    nc.scalar.activation(
                        relu[:, lo:hi], z[:, lo:hi],
                        mybir.ActivationFunctionType.Relu,
                    )
                    nc.vector.tensor_tensor(
                        hh[:, lo:hi], t1[:, lo:hi], relu[:, lo:hi],
                        mybir.AluOpType.add,
                    )
                    if lo == 0:
                        for _ in range(3):
                            nc.tensor.matmul(
                                scr[:, 0:128], ex2[:, 0:128], ex2[:, 0:128],
                                start=True, stop=True,
                            )
                for q in range(4):
                    tps = tp.tile([128, 128], dt, tag=f"t{q % 2}", bufs=1)
                    nc.tensor.transpose(
                        tps[:], hh[:, q * 128 : (q + 1) * 128], ident
                    )
                    nc.vector.tensor_tensor(
                        he_next[:, q * 1024 : (q + 1) * 1024].rearrange(
                            "p (e b) -> p e b", e=E
                        ),
                        tps[:][:, None, :].broadcast_to([128, E, 128]),
                        bb_e_b,
                        mybir.AluOpType.mult,
                    )
                    yield q

            # layer 1
            he1 = acts.tile([128, E * 512], dt, tag="he")
            z1 = zp.tile([128, D2], f32, tag="z")
            nc.tensor.matmul(z1[:], bl8, waug[1], start=True, stop=False)
            order = _order1()
            bgen = boundary(z0, he1, "b0")
            next(bgen)
            next(bgen)
            for j, (e, it) in enumerate(order):
                if j in (8, 16):
                    next(bgen)  # queue quarter q+1 behind this it-block
                g, loc = _locate(W1_GROUPS, j)
                nc.tensor.matmul(
                    z1[:],
                    he1[:, it * 1024 + e * 128 : it * 1024 + (e + 1) * 128],
                    w1[g][:, loc * D2 : (loc + 1) * D2],
                    start=False,
                    stop=(j == 31),
                )

            # layer 2
            he2 = acts.tile([128, E * 512], dt, tag="he")
            z2 = zo.tile([128, D3], f32, tag="z2")
            nc.tensor.matmul(z2[:], bl8, waug[2], start=True, stop=False)
            bgen = boundary(z1, he2, "b1")
            next(bgen)
            next(bgen)
            for j, (e, it) in enumerate(order):
                if j in (8, 16):
                    next(bgen)
                g, loc = _locate(W2_GROUPS, j)
                nc.tensor.matmul(
                    z2[:],
                    he2[:, it * 1024 + e * 128 : it * 1024 + (e + 1) * 128],
                    w2[g][:, loc * D3 : (loc + 1) * D3],
                    start=False,
                    stop=(j == 31),
                )
            out_sb = tmp.tile([128, D3], f32, tag="osb")
            nc.vector.tensor_copy(out_sb[:, 0:156], z2[:, 0:156])
            nc.scalar.copy(out_sb[:, 156:D3], z2[:, 156:D3])
            nc.sync.dma_start(out_d[:], out_sb[:])

    _split_multi_waits(nc)
    return nc


# ---- host-side packing -----------------------------------------------------


def _wimgs(w0, w1, w2, np_dt):
    wt = [np.ascontiguousarray(w.transpose(0, 2, 1)) for w in (w0, w1, w2)]

    # layer 0: it-major over (it, e), it=3 rows 96:128 zero-padded
    w0i = np.zeros((128, 32 * D1), np.float32)
    j = 0
    for it in range(4):
        for e in range(E):
            rows = 128 if it < 3 else 96
            w0i[:rows, j * D1 : (j + 1) * D1] = wt[0][e][
                it * 128 : it * 128 + rows
            ]
            j += 1

    order = _order1()
    w1i = np.empty((128, 32 * D2), np.float32)
    for j, (e, it) in enumerate(order):
        w1i[:, j * D2 : (j + 1) * D2] = wt[1][e][it * 128 : (it + 1) * 128]
    w2i = np.empty((128, 32 * D3), np.float32)
    for j, (e, it) in enumerate(order):
        w2i[:, j * D3 : (j + 1) * D3] = wt[2][e][it * 128 : (it + 1) * 128]
    return {
        "w0": np.ascontiguousarray(w0i).astype(np_dt),
        "w1": np.ascontiguousarray(w1i).astype(np_dt),
        "w2": np.ascontiguousarray(w2i).astype(np_dt),
    }


def kernel(x, weight_blend, w0, b0, w1, b1, w2, b2):
    np_dt = np.float16

    if "nc" not in _NC_CACHE:
        _NC_CACHE["nc"] = _build_nc()
    nc = _NC_CACHE["nc"]

    x = np.asarray(x, np.float32)
    weight_blend = np.asarray(weight_blend, np.float32)
    wimgs = _wimgs(np.asarray(w0), np.asarray(w1), np.asarray(w2), np_dt)
    biases = [np.asarray(b, np.float32) for b in (b0, b1, b2)]
    eye = np.eye(128, dtype=np.float32)

    bc = B // N_CORES
    in_maps = []
    for c in range(N_CORES):
        sl = slice(c * bc, (c + 1) * bc)
        xT = np.zeros((4 * 128, bc), np.float32)
        xT[:DIN] = x[sl].T
        xt_img = xT.reshape(4, 128, bc).transpose(1, 0, 2).reshape(128, 4 * bc)
        bl = weight_blend[:, sl]  # (8, 128)
        small_img = np.concatenate([bl] + biases, axis=1)
        bbsm = np.broadcast_to(bl[None, :, :], (128, E, bc)).reshape(128, E * bc)
        head = np.concatenate([xt_img, eye, bbsm], axis=1)
        in_maps.append(
            {
                **wimgs,
                "head": np.ascontiguousarray(head).astype(np_dt),
                "small": np.ascontiguousarray(small_img).astype(np_dt),
            }
        )

    res = run_bass_kernel_spmd(
        nc,
        in_maps,
        core_ids=list(range(N_CORES)),
        trace=PROFILE["trace"],
        tmpdir=PROFILE["tmpdir"],
    )
    LAST_RESULT[0] = res
    return np.concatenate(
        [res.results[c]["out"] for c in range(N_CORES)], axis=0
    )
